# revision 30
# baseline (speedup 1.0000x reference)
"""Trainium2 Bass kernel for nn_BDH_69638599737422 (dense_transformer).

Sharding (8 NeuronCores): core c = 2*h + j owns head h (of 4) and N-half j
(4096 of 8192 latent dims). encoder/encoder_v column-parallel, decoder
row-parallel. Per layer: one 2-rank AllReduce (partial yKV within a head
pair, since scores contract over the full head N) and one 8-rank AllReduce
(y = xy @ decoder partial sums into D).

All on-device tensors are fp16 (PE matmuls run fp16 at full rate with fp32
PSUM accumulation; verified ~1.3e-3 rel err vs the fp32 reference).

The RoPE frequency table repeats in pairs (quantize(t,2)), so a host-side
even/odd de-interleave permutation of each core's N slice (baked into
encoder/encoder_v columns and decoder rows) turns rotate_half into two
contiguous halves: qe = xe*c - xo*s, qo = xo*c + xe*s.

The causal mask (strict lower) is applied on the transposed score matrix
P[s,t] = scores[t,s]: Q@Q^T is symmetric, so P comes out of the same
matmuls and the mask becomes strict-upper, which lets the per-s-chunk
matmuls skip the all-zero left region entirely (triangle skip).
"""

import numpy as np

import concourse.bass as bass
import concourse.tile as tile
from concourse import bacc, mybir
from concourse.bass_utils import run_bass_kernel_spmd
from concourse.masks import make_identity

F16 = mybir.dt.float16
BF16 = mybir.dt.bfloat16
F32 = mybir.dt.float32
AF = mybir.ActivationFunctionType
ALU = mybir.AluOpType

B, T, D, NH, VOCAB = 1, 512, 256, 4, 256
N = 8192        # latent dim per head
NL = 4096       # per-core N slice
NPAIR = 2048    # rope pairs per core
NT = NL // 128  # 32 n-tiles per core
N_LAYER = 6
EPS = 1e-5
THETA = 2.0 ** 16
TWO_PI = 2.0 * np.pi
CORES = list(range(8))
PAIR_GROUPS = [[0, 1], [2, 3], [4, 5], [6, 7]]

_STATE = {}


# ---------------------------------------------------------------- host prep

def _ln_np(x):
    m = x.mean(-1, keepdims=True)
    v = ((x - m) ** 2).mean(-1, keepdims=True)
    return (x - m) / np.sqrt(v + EPS)


def _rope_pair_tables():
    """cos/sin at even lanes only (freqs repeat in pairs): [T, N//2] f32."""
    t = np.arange(N, dtype=np.float32)
    q = (np.floor(t / 2.0) * 2.0).astype(np.float32)
    freqs = (1.0 / (THETA ** (q / np.float32(N))) / np.float32(TWO_PI)).astype(
        np.float32
    )
    pos = np.arange(T, dtype=np.float32)
    ang = ((pos[:, None] * freqs[None, :]) % 1.0) * np.float32(TWO_PI)
    cos = np.cos(ang).astype(np.float32)
    sin = np.sin(ang).astype(np.float32)
    return cos[:, ::2], sin[:, ::2]


def _tileize_rows(a, rows_per_tile=128):
    """[n_tiles*128, w] -> [128, n_tiles*w] with free dim = (tile, w)."""
    r, w = a.shape
    nt = r // rows_per_tile
    return np.ascontiguousarray(
        a.reshape(nt, rows_per_tile, w).transpose(1, 0, 2).reshape(rows_per_tile, nt * w)
    )


def _build_in_maps(idx, embed, encoder, encoder_v, decoder, lm_head):
    idx = np.asarray(idx)
    embed = np.asarray(embed, dtype=np.float32)
    encoder = np.asarray(encoder, dtype=np.float32)
    encoder_v = np.asarray(encoder_v, dtype=np.float32)
    decoder = np.asarray(decoder, dtype=np.float32)
    lm_head = np.asarray(lm_head, dtype=np.float32)

    x0 = _ln_np(embed[idx[0]]).astype(np.float16)          # [T, D]
    x_td0 = _tileize_rows(x0)                               # [128, 4*256]
    x_dt0 = _tileize_rows(np.ascontiguousarray(x0.T))       # [128, 2*512]

    cos_p, sin_p = _rope_pair_tables()                      # [T, 4096] f32
    # even lanes first, then odd lanes
    perm = np.concatenate([np.arange(0, NL, 2), np.arange(1, NL, 2)])

    maskd = np.triu(np.ones((128, 128), np.float16), k=1)   # keep s < t
    lmh = _tileize_rows(lm_head.astype(np.float16))         # [128, 2*256]

    in_maps = []
    for c in CORES:
        h, j = c // 2, c % 2
        nsl = slice(j * NL, (j + 1) * NL)
        enc_s = encoder[h][:, nsl][:, perm].astype(np.float16)      # [256, 4096]
        ev_s = encoder_v[h][:, nsl][:, perm].astype(np.float16)
        dec_s = decoder[h * N + j * NL : h * N + (j + 1) * NL][perm].astype(
            np.float16
        )                                                            # [4096, 256]
        kp = slice(j * NPAIR, (j + 1) * NPAIR)
        cos_s = np.ascontiguousarray(cos_p[:, kp].T).astype(np.float16)  # [2048, 512]
        sin_s = np.ascontiguousarray(sin_p[:, kp].T).astype(np.float16)
        in_maps.append(
            {
                "enc0": np.ascontiguousarray(enc_s[:128]),
                "enc1": np.ascontiguousarray(enc_s[128:]),
                "ev0": np.ascontiguousarray(ev_s[:128]),
                "ev1": np.ascontiguousarray(ev_s[128:]),
                "decb": _tileize_rows(dec_s),               # [128, 32*256]
                "cosb": _tileize_rows(cos_s),               # [128, 16*512]
                "sinb": _tileize_rows(sin_s),
                "maskd": maskd,
                "x_td0": x_td0,
                "x_dt0": x_dt0,
                "lmh": lmh,
            }
        )
    return in_maps


# ---------------------------------------------------------------- device code

def _ln_chunk(nc, st, out_f16, in_ap, tc, chunk, epst, eng=None):
    """LN over one free-dim chunk: out = (in - mu) * rstd.

    eng selects the vector-capable engine (default DVE); sqrt is ACT."""
    if eng is None:
        eng = nc.vector
    sl = slice(tc * chunk, (tc + 1) * chunk)
    stats = st.tile([128, 6], F32, tag="st6", name="stats")
    mv = st.tile([128, 2], F32, tag="st2", name="mv")
    eng.bn_stats(out=stats, in_=in_ap[:, sl])
    eng.bn_aggr(out=mv, in_=stats)
    nc.scalar.activation(
        out=mv[:, 1:2], in_=mv[:, 1:2], func=AF.Sqrt, bias=epst, scale=1.0
    )
    eng.reciprocal(out=mv[:, 1:2], in_=mv[:, 1:2])
    eng.tensor_scalar(
        out=out_f16[:, sl],
        in0=in_ap[:, sl],
        scalar1=mv[:, 0:1],
        scalar2=mv[:, 1:2],
        op0=ALU.subtract,
        op1=ALU.mult,
    )


def _transpose_blocks(nc, ps, dst, src, n_tc, n_dc, ident):
    """dst[(dc,t-block)] = src[(tc,d-block)]^T for [128,128] blocks.

    src free = (tc, n_dc*128), dst free = (dc, n_tc*128)."""
    for tc in range(n_tc):
        for dc in range(n_dc):
            tr = ps.tile([128, 128], F16, tag="ps")
            nc.tensor.transpose(
                tr, src[:, tc * (n_dc * 128) + dc * 128 :][:, :128], ident
            )
            nc.scalar.copy(
                out=dst[:, dc * (n_tc * 128) + tc * 128 :][:, :128], in_=tr
            )


def _build_bass():
    nc = bacc.Bacc(None, target_bir_lowering=False, num_devices=len(CORES))

    dp = nc.declare_dram_parameter
    enc0_e = dp("enc0", [128, NL], F16, isOutput=False)
    enc1_e = dp("enc1", [128, NL], F16, isOutput=False)
    ev0_e = dp("ev0", [128, NL], F16, isOutput=False)
    ev1_e = dp("ev1", [128, NL], F16, isOutput=False)
    dec_e = dp("decb", [128, NT * D], F16, isOutput=False)
    cos_e = dp("cosb", [128, 16 * T], F16, isOutput=False)
    sin_e = dp("sinb", [128, 16 * T], F16, isOutput=False)
    mask_e = dp("maskd", [128, 128], F16, isOutput=False)
    xtd_e = dp("x_td0", [128, 4 * D], F16, isOutput=False)
    xdt_e = dp("x_dt0", [128, 2 * T], F16, isOutput=False)
    lmh_e = dp("lmh", [128, 2 * VOCAB], F16, isOutput=False)
    out_e = dp("logits", [T, VOCAB], F32, isOutput=True)

    with tile.TileContext(nc) as tc_:
        pools = [
            tc_.tile_pool(name="wt", bufs=1),
            tc_.tile_pool(name="big", bufs=1),
            tc_.tile_pool(name="xp", bufs=2),
            tc_.tile_pool(name="tmp", bufs=1),
            tc_.tile_pool(name="ys", bufs=3),
            tc_.tile_pool(name="st", bufs=8),
            tc_.tile_pool(name="stg", bufs=1),
            tc_.tile_pool(name="ps", bufs=8, space="PSUM"),
            tc_.tile_pool(name="dram", bufs=2, space="DRAM"),
        ]
        wt, big, xp, tmp, ysp, st, stg, ps, dram = [p.__enter__() for p in pools]
        try:
            _emit(nc, wt, big, xp, tmp, ysp, st, stg, ps, dram,
                  enc0_e, enc1_e, ev0_e, ev1_e, dec_e, cos_e, sin_e, mask_e,
                  xtd_e, xdt_e, lmh_e, out_e)
        finally:
            for p in reversed(pools):
                p.__exit__(None, None, None)
    nc.compile()
    return nc


def _emit(nc, wt, big, xp, tmp, ysp, st, stg, ps, dram,
          enc0_e, enc1_e, ev0_e, ev1_e, dec_e, cos_e, sin_e, mask_e,
          xtd_e, xdt_e, lmh_e, out_e):
    dma = nc.sync.dma_start

    # persistent weights / tables
    enc0 = wt.tile([128, NL], F16, tag="enc0")
    enc1 = wt.tile([128, NL], F16, tag="enc1")
    ev0 = wt.tile([128, NL], F16, tag="ev0")
    ev1 = wt.tile([128, NL], F16, tag="ev1")
    dect = wt.tile([128, NT * D], F16, tag="dect")
    cost = wt.tile([128, 16 * T], F16, tag="cost")
    sint = wt.tile([128, 16 * T], F16, tag="sint")
    maskt = wt.tile([128, 128], F16, tag="maskt")
    lmht = wt.tile([128, 2 * VOCAB], F16, tag="lmht")
    ident = wt.tile([128, 128], F16, tag="ident")
    epst = wt.tile([128, 1], F32, tag="epst")

    xsb = big.tile([128, NT * T], F16, tag="xsb")    # xs then xy, (i, t)
    qrb = big.tile([128, NT * T], F16, tag="qrb")    # roped qs, (i, t)
    Pb = big.tile([128, 4 * T], F16, tag="Pb")       # masked scores^T, (sc, t)

    x_first = xp.tile([128, 4 * D], F16, tag="x_td")
    xd_first = xp.tile([128, 2 * T], F16, tag="x_dt")
    dma(out=x_first, in_=xtd_e[:])
    dma(out=xd_first, in_=xdt_e[:])
    dma(out=enc0, in_=enc0_e[:])
    dma(out=enc1, in_=enc1_e[:])
    dma(out=cost, in_=cos_e[:])
    dma(out=sint, in_=sin_e[:])
    dma(out=maskt, in_=mask_e[:])
    dma(out=ev0, in_=ev0_e[:])
    dma(out=ev1, in_=ev1_e[:])
    dma(out=dect, in_=dec_e[:])
    dma(out=lmht, in_=lmh_e[:])
    nc.vector.memset(epst, EPS)
    make_identity(nc, ident[:])

    # zero regions of P (left of the diagonal block) are written once;
    # every layer only rewrites the diagonal + upper blocks
    for m in range(1, 4):
        nc.vector.memset(Pb[:, m * T : m * T + m * 128], 0.0)

    # warm up the collective path (first-call setup costs ~30us) while the
    # weight DMAs stream in; outputs are never consumed. Buffers match the
    # real per-layer collectives in shape so any size-keyed ring setup warms.
    wup = stg.tile([128, 2 * D], F16, tag="wup")
    nc.vector.memset(wup, 0.0)
    wag_i = dram.tile([128, D], F32, tag="wag_i")
    wag_o = dram.tile([2, 128, D], F32, tag="wag_o")
    war_i = dram.tile([128, 2 * D], F16, tag="war_i")
    war_o = dram.tile([128, 2 * D], F16, tag="war_o")
    dma(out=wag_i[:].bitcast(F16), in_=wup)
    dma(out=war_i, in_=wup)
    nc.gpsimd.collective_compute(
        "AllGather", ALU.bypass, replica_groups=PAIR_GROUPS,
        ins=[wag_i.opt()], outs=[wag_o.opt()],
    )
    nc.gpsimd.collective_compute(
        "AllReduce", ALU.add, replica_groups=[CORES],
        ins=[war_i.opt()], outs=[war_o.opt()],
    )

    # phase 1: xs = relu(x @ enc), out [nl, t] tiles. t-halved (so it can
    # slide into the previous layer's AR2 windows) and (even, odd) lane
    # tiles produced pairwise so rope chunks unblock early. Relu copies
    # alternate ACT/DVE to keep either from becoming the feeder bottleneck.
    order = [k for p in zip(range(16), range(16, NT)) for k in p]

    xsb_r = xsb.rearrange("p (i t) -> p i t", t=T)

    def phase1_half(h, x_dt_ap):
        hsl = slice(h * 256, h * 256 + 256)
        rhs0 = x_dt_ap[:, 0 * T : 1 * T][:, hsl]
        rhs1 = x_dt_ap[:, 1 * T : 2 * T][:, hsl]
        for n in range(16):
            mm = ps.tile([128, 512], F32, tag="ps", name="mm1")
            for k, i in enumerate((n, n + 16)):
                nc.tensor.matmul(
                    out=mm[:, k * 256 : (k + 1) * 256],
                    lhsT=enc0[:, i * 128 : (i + 1) * 128],
                    rhs=rhs0, start=True, stop=False,
                )
                nc.tensor.matmul(
                    out=mm[:, k * 256 : (k + 1) * 256],
                    lhsT=enc1[:, i * 128 : (i + 1) * 128],
                    rhs=rhs1, start=False, stop=True,
                )
            # one batched relu for the (n, n+16) tile pair via a strided AP;
            # every 4th pair goes to DVE to keep ACT from being the feeder
            # bottleneck of rope/P3
            nc.scalar.activation(
                out=xsb_r[:, n : n + 17 : 16, hsl],
                in_=mm.rearrange("p (i t) -> p i t", t=256),
                func=AF.Relu,
            )

    x_td, x_dt = x_first, xd_first
    phase1_half(0, x_dt)
    phase1_half(1, x_dt)
    for _layer in range(N_LAYER):
        # -- phase 2: rope, 4 chunks of 4 tile-pairs, all on DVE (GpSimd
        # sharing the reads poisons DVE SBUF ports — measured 4.4x slowdown).
        # qe = xe*c - xo*s, qo = xo*c + xe*s, all plain tensor_tensor (2x).
        # Full-T chunks on purpose: rope must depend on phase-1 h1 so the
        # scheduler cannot hoist it ahead of the AR-h1 LN chain in the
        # in-order DVE queue (h-split rope measured +10us/layer from that).
        CH = 4 * T  # 2048 columns per chunk
        for c in range(4):
            e_sl = slice(c * CH, (c + 1) * CH)
            o_sl = slice(16 * T + c * CH, 16 * T + (c + 1) * CH)
            tme = tmp.tile([128, CH], F16, tag="tmpe")
            nc.vector.tensor_mul(tme, xsb[:, o_sl], sint[:, e_sl])
            nc.vector.tensor_mul(qrb[:, e_sl], xsb[:, e_sl], cost[:, e_sl])
            nc.vector.tensor_sub(qrb[:, e_sl], qrb[:, e_sl], tme)
            tmo = tmp.tile([128, CH], F16, tag="tmpo")
            nc.vector.tensor_mul(tmo, xsb[:, e_sl], sint[:, e_sl])
            nc.vector.tensor_mul(qrb[:, o_sl], xsb[:, o_sl], cost[:, e_sl])
            nc.vector.tensor_add(qrb[:, o_sl], qrb[:, o_sl], tmo)

        # -- phase 3: P[s,t] = (qr^T qr) masked to s < t (triangle skip).
        # jt streamed in rope-chunk completion order so PE consumption
        # tracks DVE production. Two passes over jt: pass A accumulates
        # t-blocks {0,1}, which is all phase 4 h0 needs (strict lower
        # triangle), so the h0 pair-exchange launches while pass B
        # (t-blocks {2,3}) still runs on the PE.
        P_ps = [ps.tile([128, T], F32, tag="ps", name=f"P_ps{m}") for m in range(4)]
        jt_stream = []
        for c in range(4):
            jt_stream += [4 * c + k for k in range(4)]
            jt_stream += [16 + 4 * c + k for k in range(4)]
        ykv_ps = [ps.tile([128, D], F32, tag="ps", name=f"ykv_ps{m}") for m in range(4)]
        stage1 = stg.tile([128, 4 * D], F16, tag="stg1")
        b1o = []

        def p3_pass(ms):
            for idx_jt, jt in enumerate(jt_stream):
                base = jt * T
                for m in ms:
                    t0 = m * 128
                    nc.tensor.matmul(
                        out=P_ps[m][:, t0:T],
                        lhsT=qrb[:, base + t0 : base + t0 + 128],
                        rhs=qrb[:, base + t0 : base + T],
                        start=(idx_jt == 0), stop=(idx_jt == NT - 1),
                        skip_group_check=True,
                    )
            for m in ms:
                t0 = m * 128
                nc.vector.tensor_mul(
                    Pb[:, m * T + t0 : m * T + t0 + 128],
                    P_ps[m][:, t0 : t0 + 128],
                    maskt,
                )
                if m < 3:
                    nc.scalar.copy(
                        out=Pb[:, m * T + t0 + 128 : (m + 1) * T],
                        in_=P_ps[m][:, t0 + 128 : T],
                    )

        # -- phase 4: yKV partial = P^T-contraction with x (V); the pair
        # reduce is a bypass AllGather (fp16-safe, low floor) + local add.
        # Chunks sc > tcn of P^T are identically zero (strict lower) — skip.
        def p4_half(h):
            for k in range(2):
                tcn = 2 * h + k
                for sc in range(tcn + 1):
                    nc.tensor.matmul(
                        out=ykv_ps[tcn],
                        lhsT=Pb[:, sc * T + tcn * 128 : sc * T + (tcn + 1) * 128],
                        rhs=x_td[:, sc * D : (sc + 1) * D],
                        start=(sc == 0), stop=(sc == tcn),
                        skip_group_check=True,
                    )
                # 1/64 pre-scale keeps the pair-sum inside fp16 range; the
                # LN that follows is scale-invariant so this is exact.
                nc.vector.tensor_scalar_mul(
                    out=stage1[:, tcn * D : (tcn + 1) * D],
                    in0=ykv_ps[tcn],
                    scalar1=0.015625,
                )
            b1i_h = dram.tile([128, D], F32, tag=f"b1i{h}", name=f"b1i{h}")
            b1o_h = dram.tile(
                [2, 128, D], F32, tag=f"b1o{h}", name=f"b1o{h}"
            )
            dma(
                out=b1i_h[:].bitcast(F16),
                in_=stage1[:, h * 2 * D : (h + 1) * 2 * D],
            )
            nc.gpsimd.collective_compute(
                "AllGather", ALU.bypass, replica_groups=PAIR_GROUPS,
                ins=[b1i_h.opt()], outs=[b1o_h.opt()],
            )
            b1o.append(b1o_h)

        p3_pass((0, 1))
        p4_half(0)
        p3_pass((2, 3))
        p4_half(1)

        ykvsum = stg.tile([128, 4 * D], F16, tag="ykvsum")
        agt = stg.tile([128, 2 * D], F16, tag="agt")
        ykv_td = xp.tile([128, 4 * D], F16, tag="ykv_td")
        ykv_dt = xp.tile([128, 2 * T], F16, tag="ykv_dt")
        stage2 = stg.tile([128, 4 * D], F16, tag="stg2")
        b2o = []
        for h in range(2):
            hsl = slice(h * 2 * D, (h + 1) * 2 * D)
            # per-half y-partial accumulators (2 banks held instead of 4 —
            # holding 4 across all of phase 5 capped the mm5 pipeline depth;
            # NOTE: they must be separate tiles — two interleaved
            # accumulation groups inside one PSUM bank corrupt results)
            y_ps_h = [
                ps.tile([128, D], F32, tag="ps", name=f"y_ps{h}{k}")
                for k in range(2)
            ]
            dma(out=ykvsum[:, hsl], in_=b1o[h][0].bitcast(F16))
            dma(out=agt, in_=b1o[h][1].bitcast(F16))
            nc.vector.tensor_add(ykvsum[:, hsl], ykvsum[:, hsl], agt)
            for k in range(2):
                tcn = 2 * h + k
                _ln_chunk(nc, st, ykv_td, ykvsum, tcn, D, epst)
                for dc in range(2):
                    tr = ps.tile([128, 128], F16, tag="ps", name="tr")
                    nc.tensor.transpose(
                        tr, ykv_td[:, tcn * D + dc * 128 :][:, :128], ident
                    )
                    nc.scalar.copy(
                        out=ykv_dt[:, dc * T + tcn * 128 :][:, :128], in_=tr
                    )

            # phase 5 half: ys = relu(yKV @ encv); xy = xs*ys; y += xy @ dec.
            # xy runs once per 4-tile group through a strided 3D AP to
            # amortize DVE op overhead. The dec matmuls trail the ys stream
            # by 2 groups so the relu->xy cross-engine latency (~1.4us of
            # semaphore hops) is hidden behind later ys groups.
            NG = NT // 4

            def ys_group(g):
                ys_grp = ysp.tile([128, 4 * 256], F16, tag="ys", name="ys_grp")
                for h2 in range(2):
                    mm = ps.tile([128, 512], F32, tag="ps", name="mm5")
                    for k2 in range(2):
                        i = 4 * g + 2 * h2 + k2
                        nc.tensor.matmul(
                            out=mm[:, k2 * 256 : (k2 + 1) * 256],
                            lhsT=ev0[:, i * 128 : (i + 1) * 128],
                            rhs=ykv_dt[:, 0 * T + h * 256 :][:, :256],
                            start=True, stop=False,
                        )
                        nc.tensor.matmul(
                            out=mm[:, k2 * 256 : (k2 + 1) * 256],
                            lhsT=ev1[:, i * 128 : (i + 1) * 128],
                            rhs=ykv_dt[:, 1 * T + h * 256 :][:, :256],
                            start=False, stop=True,
                        )
                    nc.scalar.activation(
                        out=ys_grp[:, h2 * 512 : (h2 + 1) * 512], in_=mm,
                        func=AF.Relu,
                    )
                xs_grp = (
                    xsb[:, 4 * g * T : 4 * (g + 1) * T]
                    .rearrange("p (i t) -> p i t", t=T)[:, :, h * 256 : (h + 1) * 256]
                )
                nc.vector.tensor_mul(
                    xs_grp, xs_grp,
                    ys_grp.rearrange("p (i t) -> p i t", t=256),
                )

            def dec_group(g):
                for k4 in range(4):
                    i = 4 * g + k4
                    for k in range(2):
                        tcn = 2 * h + k
                        nc.tensor.matmul(
                            out=y_ps_h[k],
                            lhsT=xsb[:, i * T + tcn * 128 : i * T + (tcn + 1) * 128],
                            rhs=dect[:, i * D : (i + 1) * D],
                            start=(i == 0), stop=(i == NT - 1),
                            skip_group_check=True,
                        )

            for g in range(NG + 2):
                if g < NG:
                    ys_group(g)
                if g >= 2:
                    dec_group(g - 2)

            # 8-core AllReduce of this half's y partials
            for k in range(2):
                nc.vector.tensor_copy(
                    out=stage2[:, (2 * h + k) * D : (2 * h + k + 1) * D],
                    in_=y_ps_h[k],
                )
            b2i_h = dram.tile([128, 2 * D], F16, tag=f"b2i{h}", name=f"b2i{h}")
            b2o_h = dram.tile([128, 2 * D], F16, tag=f"b2o{h}", name=f"b2o{h}")
            dma(out=b2i_h, in_=stage2[:, hsl])
            nc.gpsimd.collective_compute(
                "AllReduce", ALU.add, replica_groups=[CORES],
                ins=[b2i_h.opt()], outs=[b2o_h.opt()],
            )
            b2o.append(b2o_h)

        # -- phase 7: y = LN(ysum); x = LN(x + y); refresh x_dt (per half)
        ysum = stg.tile([128, 4 * D], F16, tag="ysum")
        y_ln = xp.tile([128, 4 * D], F16, tag="y_ln")
        z = xp.tile([128, 4 * D], F16, tag="z")
        x_td_new = xp.tile([128, 4 * D], F16, tag="x_td")
        x_dt_new = xp.tile([128, 2 * T], F16, tag="x_dt")
        for h in range(2):
            hsl = slice(h * 2 * D, (h + 1) * 2 * D)
            dma(out=ysum[:, hsl], in_=b2o[h])
            for k in range(2):
                tcn = 2 * h + k
                _ln_chunk(nc, st, y_ln, ysum, tcn, D, epst)
            nc.vector.tensor_add(z[:, hsl], y_ln[:, hsl], x_td[:, hsl])
            for k in range(2):
                tcn = 2 * h + k
                _ln_chunk(nc, st, x_td_new, z, tcn, D, epst)
                for dc in range(2):
                    tr = ps.tile([128, 128], F16, tag="ps", name="trx")
                    nc.tensor.transpose(
                        tr, x_td_new[:, tcn * D + dc * 128 :][:, :128], ident
                    )
                    nc.scalar.copy(
                        out=x_dt_new[:, dc * T + tcn * 128 :][:, :128], in_=tr
                    )
            # next layer's phase-1 half rides in this AR2/tail window
            if _layer < N_LAYER - 1:
                phase1_half(h, x_dt_new)
            else:
                for k in range(2):
                    tcn = 2 * h + k
                    lg = ps.tile([128, VOCAB], F32, tag="ps", name="lg")
                    for dc in range(2):
                        nc.tensor.matmul(
                            out=lg,
                            lhsT=x_dt_new[:, dc * T + tcn * 128 : dc * T + (tcn + 1) * 128],
                            rhs=lmht[:, dc * VOCAB : (dc + 1) * VOCAB],
                            start=(dc == 0), stop=(dc == 1),
                        )
                    lg_sb = ysp.tile([128, VOCAB], F32, tag="lg", name="lg_sb")
                    nc.vector.tensor_copy(out=lg_sb, in_=lg)
                    dma(out=out_e[tcn * 128 : (tcn + 1) * 128, :], in_=lg_sb)
        x_td, x_dt = x_td_new, x_dt_new



# ---------------------------------------------------------------- entry point

def kernel(idx, embed, encoder, encoder_v, decoder, lm_head):
    if "nc" not in _STATE:
        _STATE["nc"] = _build_bass()
    nc = _STATE["nc"]
    in_maps = _build_in_maps(idx, embed, encoder, encoder_v, decoder, lm_head)
    import os

    trace = bool(int(os.environ.get("KERNEL_TRACE", "0")))
    res = run_bass_kernel_spmd(nc, in_maps, core_ids=CORES, trace=trace)
    _STATE["last_results"] = res
    return res.results[0]["logits"].reshape(B, T, VOCAB).astype(np.float32)



# revision 32
# speedup vs baseline: 1.1058x; 1.1058x over previous
"""Trainium2 Bass kernel for nn_BDH_69638599737422 (dense_transformer).

Sharding (8 NeuronCores): core c = 2*h + j owns head h (of 4) and N-half j
(4096 of 8192 latent dims). encoder/encoder_v column-parallel, decoder
row-parallel. Per layer: one 2-rank AllReduce (partial yKV within a head
pair, since scores contract over the full head N) and one 8-rank AllReduce
(y = xy @ decoder partial sums into D).

All on-device tensors are fp16 (PE matmuls run fp16 at full rate with fp32
PSUM accumulation; verified ~1.3e-3 rel err vs the fp32 reference).

The RoPE frequency table repeats in pairs (quantize(t,2)), so a host-side
even/odd de-interleave permutation of each core's N slice (baked into
encoder/encoder_v columns and decoder rows) turns rotate_half into two
contiguous halves: qe = xe*c - xo*s, qo = xo*c + xe*s.

The causal mask (strict lower) is applied on the transposed score matrix
P[s,t] = scores[t,s]: Q@Q^T is symmetric, so P comes out of the same
matmuls and the mask becomes strict-upper, which lets the per-s-chunk
matmuls skip the all-zero left region entirely (triangle skip).
"""

import numpy as np

import concourse.bass as bass
import concourse.tile as tile
from concourse import bacc, mybir
from concourse.bass_utils import run_bass_kernel_spmd
from concourse.masks import make_identity

F16 = mybir.dt.float16
BF16 = mybir.dt.bfloat16
F32 = mybir.dt.float32
AF = mybir.ActivationFunctionType
ALU = mybir.AluOpType

B, T, D, NH, VOCAB = 1, 512, 256, 4, 256
N = 8192        # latent dim per head
NL = 4096       # per-core N slice
NPAIR = 2048    # rope pairs per core
NT = NL // 128  # 32 n-tiles per core
N_LAYER = 6
EPS = 1e-5
THETA = 2.0 ** 16
TWO_PI = 2.0 * np.pi
CORES = list(range(8))
PAIR_GROUPS = [[0, 1], [2, 3], [4, 5], [6, 7]]

_STATE = {}


# ---------------------------------------------------------------- host prep

def _ln_np(x):
    m = x.mean(-1, keepdims=True)
    v = ((x - m) ** 2).mean(-1, keepdims=True)
    return (x - m) / np.sqrt(v + EPS)


def _rope_pair_tables():
    """cos/sin at even lanes only (freqs repeat in pairs): [T, N//2] f32."""
    t = np.arange(N, dtype=np.float32)
    q = (np.floor(t / 2.0) * 2.0).astype(np.float32)
    freqs = (1.0 / (THETA ** (q / np.float32(N))) / np.float32(TWO_PI)).astype(
        np.float32
    )
    pos = np.arange(T, dtype=np.float32)
    ang = ((pos[:, None] * freqs[None, :]) % 1.0) * np.float32(TWO_PI)
    cos = np.cos(ang).astype(np.float32)
    sin = np.sin(ang).astype(np.float32)
    return cos[:, ::2], sin[:, ::2]


def _tileize_rows(a, rows_per_tile=128):
    """[n_tiles*128, w] -> [128, n_tiles*w] with free dim = (tile, w)."""
    r, w = a.shape
    nt = r // rows_per_tile
    return np.ascontiguousarray(
        a.reshape(nt, rows_per_tile, w).transpose(1, 0, 2).reshape(rows_per_tile, nt * w)
    )


def _build_in_maps(idx, embed, encoder, encoder_v, decoder, lm_head):
    idx = np.asarray(idx)
    embed = np.asarray(embed, dtype=np.float32)
    encoder = np.asarray(encoder, dtype=np.float32)
    encoder_v = np.asarray(encoder_v, dtype=np.float32)
    decoder = np.asarray(decoder, dtype=np.float32)
    lm_head = np.asarray(lm_head, dtype=np.float32)

    x0 = _ln_np(embed[idx[0]]).astype(np.float16)          # [T, D]
    x_td0 = _tileize_rows(x0)                               # [128, 4*256]
    x_dt0 = _tileize_rows(np.ascontiguousarray(x0.T))       # [128, 2*512]

    cos_p, sin_p = _rope_pair_tables()                      # [T, 4096] f32
    # even lanes first, then odd lanes
    perm = np.concatenate([np.arange(0, NL, 2), np.arange(1, NL, 2)])

    maskd = np.triu(np.ones((128, 128), np.float16), k=1)   # keep s < t
    lmh = _tileize_rows(lm_head.astype(np.float16))         # [128, 2*256]

    in_maps = []
    for c in CORES:
        h, j = c // 2, c % 2
        nsl = slice(j * NL, (j + 1) * NL)
        enc_s = encoder[h][:, nsl][:, perm].astype(np.float16)      # [256, 4096]
        ev_s = encoder_v[h][:, nsl][:, perm].astype(np.float16)
        dec_s = decoder[h * N + j * NL : h * N + (j + 1) * NL][perm].astype(
            np.float16
        )                                                            # [4096, 256]
        kp = slice(j * NPAIR, (j + 1) * NPAIR)
        cos_s = np.ascontiguousarray(cos_p[:, kp].T).astype(np.float16)  # [2048, 512]
        sin_s = np.ascontiguousarray(sin_p[:, kp].T).astype(np.float16)
        in_maps.append(
            {
                "enc0": np.ascontiguousarray(enc_s[:128]),
                "enc1": np.ascontiguousarray(enc_s[128:]),
                "ev0": np.ascontiguousarray(ev_s[:128]),
                "ev1": np.ascontiguousarray(ev_s[128:]),
                "decb": _tileize_rows(dec_s),               # [128, 32*256]
                "cosb": _tileize_rows(cos_s),               # [128, 16*512]
                "sinb": _tileize_rows(sin_s),
                "maskd": maskd,
                "x_td0": x_td0,
                "x_dt0": x_dt0,
                "lmh": lmh,
            }
        )
    return in_maps


# ---------------------------------------------------------------- device code

def _ln_chunk(nc, st, out_f16, in_ap, tc, chunk, epst, eng=None):
    """LN over one free-dim chunk: out = (in - mu) * rstd.

    eng selects the vector-capable engine (default DVE); sqrt is ACT."""
    if eng is None:
        eng = nc.vector
    sl = slice(tc * chunk, (tc + 1) * chunk)
    stats = st.tile([128, 6], F32, tag="st6", name="stats")
    mv = st.tile([128, 2], F32, tag="st2", name="mv")
    eng.bn_stats(out=stats, in_=in_ap[:, sl])
    eng.bn_aggr(out=mv, in_=stats)
    nc.scalar.activation(
        out=mv[:, 1:2], in_=mv[:, 1:2], func=AF.Sqrt, bias=epst, scale=1.0
    )
    eng.reciprocal(out=mv[:, 1:2], in_=mv[:, 1:2])
    eng.tensor_scalar(
        out=out_f16[:, sl],
        in0=in_ap[:, sl],
        scalar1=mv[:, 0:1],
        scalar2=mv[:, 1:2],
        op0=ALU.subtract,
        op1=ALU.mult,
    )


def _transpose_blocks(nc, ps, dst, src, n_tc, n_dc, ident):
    """dst[(dc,t-block)] = src[(tc,d-block)]^T for [128,128] blocks.

    src free = (tc, n_dc*128), dst free = (dc, n_tc*128)."""
    for tc in range(n_tc):
        for dc in range(n_dc):
            tr = ps.tile([128, 128], F16, tag="ps")
            nc.tensor.transpose(
                tr, src[:, tc * (n_dc * 128) + dc * 128 :][:, :128], ident
            )
            nc.scalar.copy(
                out=dst[:, dc * (n_tc * 128) + tc * 128 :][:, :128], in_=tr
            )


def _build_bass():
    nc = bacc.Bacc(None, target_bir_lowering=False, num_devices=len(CORES))

    dp = nc.declare_dram_parameter
    enc0_e = dp("enc0", [128, NL], F16, isOutput=False)
    enc1_e = dp("enc1", [128, NL], F16, isOutput=False)
    ev0_e = dp("ev0", [128, NL], F16, isOutput=False)
    ev1_e = dp("ev1", [128, NL], F16, isOutput=False)
    dec_e = dp("decb", [128, NT * D], F16, isOutput=False)
    cos_e = dp("cosb", [128, 16 * T], F16, isOutput=False)
    sin_e = dp("sinb", [128, 16 * T], F16, isOutput=False)
    mask_e = dp("maskd", [128, 128], F16, isOutput=False)
    xtd_e = dp("x_td0", [128, 4 * D], F16, isOutput=False)
    xdt_e = dp("x_dt0", [128, 2 * T], F16, isOutput=False)
    lmh_e = dp("lmh", [128, 2 * VOCAB], F16, isOutput=False)
    out_e = dp("logits", [T, VOCAB], F32, isOutput=True)

    with tile.TileContext(nc) as tc_:
        pools = [
            tc_.tile_pool(name="wt", bufs=1),
            tc_.tile_pool(name="big", bufs=1),
            tc_.tile_pool(name="xp", bufs=2),
            tc_.tile_pool(name="tmp", bufs=1),
            tc_.tile_pool(name="ys", bufs=3),
            tc_.tile_pool(name="st", bufs=8),
            tc_.tile_pool(name="stg", bufs=1),
            tc_.tile_pool(name="ps", bufs=8, space="PSUM"),
            tc_.tile_pool(name="dram", bufs=2, space="DRAM"),
        ]
        wt, big, xp, tmp, ysp, st, stg, ps, dram = [p.__enter__() for p in pools]
        try:
            _emit(nc, wt, big, xp, tmp, ysp, st, stg, ps, dram,
                  enc0_e, enc1_e, ev0_e, ev1_e, dec_e, cos_e, sin_e, mask_e,
                  xtd_e, xdt_e, lmh_e, out_e)
        finally:
            for p in reversed(pools):
                p.__exit__(None, None, None)
    nc.compile()
    return nc


def _emit(nc, wt, big, xp, tmp, ysp, st, stg, ps, dram,
          enc0_e, enc1_e, ev0_e, ev1_e, dec_e, cos_e, sin_e, mask_e,
          xtd_e, xdt_e, lmh_e, out_e):
    dma = nc.sync.dma_start

    # persistent weights / tables
    enc0 = wt.tile([128, NL], F16, tag="enc0")
    enc1 = wt.tile([128, NL], F16, tag="enc1")
    ev0 = wt.tile([128, NL], F16, tag="ev0")
    ev1 = wt.tile([128, NL], F16, tag="ev1")
    dect = wt.tile([128, NT * D], F16, tag="dect")
    cost = wt.tile([128, 16 * T], F16, tag="cost")
    sint = wt.tile([128, 16 * T], F16, tag="sint")
    maskt = wt.tile([128, 128], F16, tag="maskt")
    lmht = wt.tile([128, 2 * VOCAB], F16, tag="lmht")
    ident = wt.tile([128, 128], F16, tag="ident")
    epst = wt.tile([128, 1], F32, tag="epst")

    xsb = big.tile([128, NT * T], F16, tag="xsb")    # xs then xy, (i, t)
    qrb = big.tile([128, NT * T], F16, tag="qrb")    # roped qs, (i, t)
    Pb = big.tile([128, 4 * T], F16, tag="Pb")       # masked scores^T, (sc, t)

    x_first = xp.tile([128, 4 * D], F16, tag="x_td")
    xd_first = xp.tile([128, 2 * T], F16, tag="x_dt")
    dma(out=x_first, in_=xtd_e[:])
    dma(out=xd_first, in_=xdt_e[:])
    dma(out=enc0, in_=enc0_e[:])
    dma(out=enc1, in_=enc1_e[:])
    dma(out=cost, in_=cos_e[:])
    dma(out=sint, in_=sin_e[:])
    dma(out=maskt, in_=mask_e[:])
    dma(out=ev0, in_=ev0_e[:])
    dma(out=ev1, in_=ev1_e[:])
    dma(out=dect, in_=dec_e[:])
    dma(out=lmht, in_=lmh_e[:])
    nc.vector.memset(epst, EPS)
    make_identity(nc, ident[:])

    # zero regions of P (left of the diagonal block) are written once;
    # every layer only rewrites the diagonal + upper blocks
    for m in range(1, 4):
        nc.vector.memset(Pb[:, m * T : m * T + m * 128], 0.0)

    # warm up the collective path (first-call setup costs ~30us) while the
    # weight DMAs stream in; outputs are never consumed. Buffers match the
    # real per-layer collectives in shape so any size-keyed ring setup warms.
    wup = stg.tile([128, 2 * D], F16, tag="wup")
    nc.vector.memset(wup, 0.0)
    wag_i = dram.tile([128, D], F32, tag="wag_i")
    wag_o = dram.tile([2, 128, D], F32, tag="wag_o")
    war_i = dram.tile([128, 2 * D], F16, tag="war_i")
    war_o = dram.tile([128, 2 * D], F16, tag="war_o")
    dma(out=wag_i[:].bitcast(F16), in_=wup)
    dma(out=war_i, in_=wup)
    nc.gpsimd.collective_compute(
        "AllGather", ALU.bypass, replica_groups=PAIR_GROUPS,
        ins=[wag_i.opt()], outs=[wag_o.opt()],
    )
    nc.gpsimd.collective_compute(
        "AllReduce", ALU.add, replica_groups=[CORES],
        ins=[war_i.opt()], outs=[war_o.opt()],
    )

    # phase 1: xs = relu(x @ enc), out [nl, t] tiles. t-halved (so it can
    # slide into the previous layer's AR2 windows) and (even, odd) lane
    # tiles produced pairwise so rope chunks unblock early. Relu copies
    # alternate ACT/DVE to keep either from becoming the feeder bottleneck.
    order = [k for p in zip(range(16), range(16, NT)) for k in p]

    xsb_r = xsb.rearrange("p (i t) -> p i t", t=T)

    def phase1_half(h, x_dt_ap):
        hsl = slice(h * 256, h * 256 + 256)
        rhs0 = x_dt_ap[:, 0 * T : 1 * T][:, hsl]
        rhs1 = x_dt_ap[:, 1 * T : 2 * T][:, hsl]
        for n in range(16):
            mm = ps.tile([128, 512], F32, tag="ps", name="mm1")
            for k, i in enumerate((n, n + 16)):
                nc.tensor.matmul(
                    out=mm[:, k * 256 : (k + 1) * 256],
                    lhsT=enc0[:, i * 128 : (i + 1) * 128],
                    rhs=rhs0, start=True, stop=False,
                )
                nc.tensor.matmul(
                    out=mm[:, k * 256 : (k + 1) * 256],
                    lhsT=enc1[:, i * 128 : (i + 1) * 128],
                    rhs=rhs1, start=False, stop=True,
                )
            # one batched relu for the (n, n+16) tile pair via a strided AP;
            # every 4th pair goes to DVE to keep ACT from being the feeder
            # bottleneck of rope/P3
            nc.scalar.activation(
                out=xsb_r[:, n : n + 17 : 16, hsl],
                in_=mm.rearrange("p (i t) -> p i t", t=256),
                func=AF.Relu,
            )

    x_td, x_dt = x_first, xd_first
    phase1_half(0, x_dt)
    phase1_half(1, x_dt)
    for _layer in range(N_LAYER):
        # -- phase 2: rope, 4 chunks of 4 tile-pairs, all on DVE (GpSimd
        # sharing the reads poisons DVE SBUF ports — measured 4.4x slowdown).
        # qe = xe*c - xo*s, qo = xo*c + xe*s, all plain tensor_tensor (2x).
        # Full-T chunks on purpose: rope must depend on phase-1 h1 so the
        # scheduler cannot hoist it ahead of the AR-h1 LN chain in the
        # in-order DVE queue (h-split rope measured +10us/layer from that).
        CH = 4 * T  # 2048 columns per chunk
        for c in range(4):
            e_sl = slice(c * CH, (c + 1) * CH)
            o_sl = slice(16 * T + c * CH, 16 * T + (c + 1) * CH)
            tme = tmp.tile([128, CH], F16, tag="tmpe")
            nc.vector.tensor_mul(tme, xsb[:, o_sl], sint[:, e_sl])
            nc.vector.tensor_mul(qrb[:, e_sl], xsb[:, e_sl], cost[:, e_sl])
            nc.vector.tensor_sub(qrb[:, e_sl], qrb[:, e_sl], tme)
            tmo = tmp.tile([128, CH], F16, tag="tmpo")
            nc.vector.tensor_mul(tmo, xsb[:, e_sl], sint[:, e_sl])
            nc.vector.tensor_mul(qrb[:, o_sl], xsb[:, o_sl], cost[:, e_sl])
            nc.vector.tensor_add(qrb[:, o_sl], qrb[:, o_sl], tmo)

        # -- phase 3: P[s,t] = (qr^T qr) masked to s < t (triangle skip).
        # jt streamed in rope-chunk completion order so PE consumption
        # tracks DVE production. Two passes over jt: pass A accumulates
        # t-blocks {0,1}, which is all phase 4 h0 needs (strict lower
        # triangle), so the h0 pair-exchange launches while pass B
        # (t-blocks {2,3}) still runs on the PE.
        P_ps = [ps.tile([128, T], F32, tag="ps", name=f"P_ps{m}") for m in range(4)]
        jt_stream = []
        for c in range(4):
            jt_stream += [4 * c + k for k in range(4)]
            jt_stream += [16 + 4 * c + k for k in range(4)]
        ykv_ps = [ps.tile([128, D], F32, tag="ps", name=f"ykv_ps{m}") for m in range(4)]
        stage1 = stg.tile([128, 4 * D], F16, tag="stg1")
        b1o = []

        def p3_pass(ms):
            for idx_jt, jt in enumerate(jt_stream):
                base = jt * T
                for m in ms:
                    t0 = m * 128
                    nc.tensor.matmul(
                        out=P_ps[m][:, t0:T],
                        lhsT=qrb[:, base + t0 : base + t0 + 128],
                        rhs=qrb[:, base + t0 : base + T],
                        start=(idx_jt == 0), stop=(idx_jt == NT - 1),
                        skip_group_check=True,
                    )
            for m in ms:
                t0 = m * 128
                nc.vector.tensor_mul(
                    Pb[:, m * T + t0 : m * T + t0 + 128],
                    P_ps[m][:, t0 : t0 + 128],
                    maskt,
                )
                if m < 3:
                    nc.scalar.copy(
                        out=Pb[:, m * T + t0 + 128 : (m + 1) * T],
                        in_=P_ps[m][:, t0 + 128 : T],
                    )

        # -- phase 4: yKV partial = P^T-contraction with x (V); the pair
        # reduce is a bypass AllGather (fp16-safe, low floor) + local add.
        # Chunks sc > tcn of P^T are identically zero (strict lower) — skip.
        def p4_half(h):
            for k in range(2):
                tcn = 2 * h + k
                for sc in range(tcn + 1):
                    nc.tensor.matmul(
                        out=ykv_ps[tcn],
                        lhsT=Pb[:, sc * T + tcn * 128 : sc * T + (tcn + 1) * 128],
                        rhs=x_td[:, sc * D : (sc + 1) * D],
                        start=(sc == 0), stop=(sc == tcn),
                        skip_group_check=True,
                    )
                # 1/64 pre-scale keeps the pair-sum inside fp16 range; the
                # LN that follows is scale-invariant so this is exact.
                nc.vector.tensor_scalar_mul(
                    out=stage1[:, tcn * D : (tcn + 1) * D],
                    in0=ykv_ps[tcn],
                    scalar1=0.015625,
                )
            b1i_h = dram.tile([128, D], F32, tag=f"b1i{h}", name=f"b1i{h}")
            b1o_h = dram.tile(
                [2, 128, D], F32, tag=f"b1o{h}", name=f"b1o{h}"
            )
            dma(
                out=b1i_h[:].bitcast(F16),
                in_=stage1[:, h * 2 * D : (h + 1) * 2 * D],
            )
            nc.gpsimd.collective_compute(
                "AllGather", ALU.bypass, replica_groups=PAIR_GROUPS,
                ins=[b1i_h.opt()], outs=[b1o_h.opt()],
            )
            b1o.append(b1o_h)

        p3_pass((0, 1))
        p4_half(0)
        p3_pass((2, 3))
        p4_half(1)

        ykvsum = stg.tile([128, 4 * D], F16, tag="ykvsum")
        agt = stg.tile([128, 2 * D], F16, tag="agt")
        ykv_td = xp.tile([128, 4 * D], F16, tag="ykv_td")
        ykv_dt = xp.tile([128, 2 * T], F16, tag="ykv_dt")
        stage2 = stg.tile([128, 4 * D], F16, tag="stg2")
        b2o = []
        for h in range(2):
            hsl = slice(h * 2 * D, (h + 1) * 2 * D)
            # per-half y-partial accumulators (2 banks held instead of 4 —
            # holding 4 across all of phase 5 capped the mm5 pipeline depth;
            # NOTE: they must be separate tiles — two interleaved
            # accumulation groups inside one PSUM bank corrupt results)
            y_ps_h = [
                ps.tile([128, D], F32, tag="ps", name=f"y_ps{h}{k}")
                for k in range(2)
            ]
            dma(out=ykvsum[:, hsl], in_=b1o[h][0].bitcast(F16))
            dma(out=agt, in_=b1o[h][1].bitcast(F16))
            nc.vector.tensor_add(ykvsum[:, hsl], ykvsum[:, hsl], agt)
            for k in range(2):
                tcn = 2 * h + k
                _ln_chunk(nc, st, ykv_td, ykvsum, tcn, D, epst)
                for dc in range(2):
                    tr = ps.tile([128, 128], F16, tag="ps", name="tr")
                    nc.tensor.transpose(
                        tr, ykv_td[:, tcn * D + dc * 128 :][:, :128], ident
                    )
                    nc.scalar.copy(
                        out=ykv_dt[:, dc * T + tcn * 128 :][:, :128], in_=tr
                    )

            # phase 5 half: ys = relu(yKV @ encv); xy = xs*ys; y += xy @ dec.
            # xy runs once per 4-tile group through a strided 3D AP to
            # amortize DVE op overhead. The dec matmuls trail the ys stream
            # by 2 groups so the relu->xy cross-engine latency (~1.4us of
            # semaphore hops) is hidden behind later ys groups.
            NG = NT // 4

            def ys_group(g):
                # xy = relu(ys_raw) * xs fused into one DVE op straight from
                # PSUM: drops the ACT relu, the ys staging tile, and one
                # cross-engine semaphore hop from the dec feeder chain.
                for h2 in range(2):
                    mm = ps.tile([128, 512], F32, tag="ps", name="mm5")
                    for k2 in range(2):
                        i = 4 * g + 2 * h2 + k2
                        nc.tensor.matmul(
                            out=mm[:, k2 * 256 : (k2 + 1) * 256],
                            lhsT=ev0[:, i * 128 : (i + 1) * 128],
                            rhs=ykv_dt[:, 0 * T + h * 256 :][:, :256],
                            start=True, stop=False,
                        )
                        nc.tensor.matmul(
                            out=mm[:, k2 * 256 : (k2 + 1) * 256],
                            lhsT=ev1[:, i * 128 : (i + 1) * 128],
                            rhs=ykv_dt[:, 1 * T + h * 256 :][:, :256],
                            start=False, stop=True,
                        )
                    i0 = 4 * g + 2 * h2
                    xs_pair = xsb_r[:, i0 : i0 + 2, h * 256 : (h + 1) * 256]
                    nc.vector.scalar_tensor_tensor(
                        out=xs_pair,
                        in0=mm.rearrange("p (i t) -> p i t", t=256),
                        scalar=0.0, in1=xs_pair,
                        op0=ALU.max, op1=ALU.mult,
                    )

            def dec_group(g):
                for k4 in range(4):
                    i = 4 * g + k4
                    for k in range(2):
                        tcn = 2 * h + k
                        nc.tensor.matmul(
                            out=y_ps_h[k],
                            lhsT=xsb[:, i * T + tcn * 128 : i * T + (tcn + 1) * 128],
                            rhs=dect[:, i * D : (i + 1) * D],
                            start=(i == 0), stop=(i == NT - 1),
                            skip_group_check=True,
                        )

            # NOTE: a 2-group dec stagger was tried here and measured
            # +9us/layer — running ys ahead holds more mm5 PSUM banks
            # concurrently, so the ys stream blocks on ACT relu instead.
            for g in range(NG):
                ys_group(g)
                dec_group(g)

            # 8-core AllReduce of this half's y partials
            for k in range(2):
                nc.vector.tensor_copy(
                    out=stage2[:, (2 * h + k) * D : (2 * h + k + 1) * D],
                    in_=y_ps_h[k],
                )
            b2i_h = dram.tile([128, 2 * D], F16, tag=f"b2i{h}", name=f"b2i{h}")
            b2o_h = dram.tile([128, 2 * D], F16, tag=f"b2o{h}", name=f"b2o{h}")
            dma(out=b2i_h, in_=stage2[:, hsl])
            nc.gpsimd.collective_compute(
                "AllReduce", ALU.add, replica_groups=[CORES],
                ins=[b2i_h.opt()], outs=[b2o_h.opt()],
            )
            b2o.append(b2o_h)

        # -- phase 7: y = LN(ysum); x = LN(x + y); refresh x_dt (per half)
        ysum = stg.tile([128, 4 * D], F16, tag="ysum")
        y_ln = xp.tile([128, 4 * D], F16, tag="y_ln")
        z = xp.tile([128, 4 * D], F16, tag="z")
        x_td_new = xp.tile([128, 4 * D], F16, tag="x_td")
        x_dt_new = xp.tile([128, 2 * T], F16, tag="x_dt")
        for h in range(2):
            hsl = slice(h * 2 * D, (h + 1) * 2 * D)
            dma(out=ysum[:, hsl], in_=b2o[h])
            for k in range(2):
                tcn = 2 * h + k
                _ln_chunk(nc, st, y_ln, ysum, tcn, D, epst)
            nc.vector.tensor_add(z[:, hsl], y_ln[:, hsl], x_td[:, hsl])
            for k in range(2):
                tcn = 2 * h + k
                _ln_chunk(nc, st, x_td_new, z, tcn, D, epst)
                for dc in range(2):
                    tr = ps.tile([128, 128], F16, tag="ps", name="trx")
                    nc.tensor.transpose(
                        tr, x_td_new[:, tcn * D + dc * 128 :][:, :128], ident
                    )
                    nc.scalar.copy(
                        out=x_dt_new[:, dc * T + tcn * 128 :][:, :128], in_=tr
                    )
            # next layer's phase-1 half rides in this AR2/tail window
            if _layer < N_LAYER - 1:
                phase1_half(h, x_dt_new)
            else:
                for k in range(2):
                    tcn = 2 * h + k
                    lg = ps.tile([128, VOCAB], F32, tag="ps", name="lg")
                    for dc in range(2):
                        nc.tensor.matmul(
                            out=lg,
                            lhsT=x_dt_new[:, dc * T + tcn * 128 : dc * T + (tcn + 1) * 128],
                            rhs=lmht[:, dc * VOCAB : (dc + 1) * VOCAB],
                            start=(dc == 0), stop=(dc == 1),
                        )
                    lg_sb = ysp.tile([128, VOCAB], F32, tag="lg", name="lg_sb")
                    nc.vector.tensor_copy(out=lg_sb, in_=lg)
                    dma(out=out_e[tcn * 128 : (tcn + 1) * 128, :], in_=lg_sb)
        x_td, x_dt = x_td_new, x_dt_new



# ---------------------------------------------------------------- entry point

def kernel(idx, embed, encoder, encoder_v, decoder, lm_head):
    if "nc" not in _STATE:
        _STATE["nc"] = _build_bass()
    nc = _STATE["nc"]
    in_maps = _build_in_maps(idx, embed, encoder, encoder_v, decoder, lm_head)
    import os

    trace = bool(int(os.environ.get("KERNEL_TRACE", "0")))
    res = run_bass_kernel_spmd(nc, in_maps, core_ids=CORES, trace=trace)
    _STATE["last_results"] = res
    return res.results[0]["logits"].reshape(B, T, VOCAB).astype(np.float32)



# revision 34
# speedup vs baseline: 1.1263x; 1.0185x over previous
"""Trainium2 Bass kernel for nn_BDH_69638599737422 (dense_transformer).

Sharding (8 NeuronCores): core c = 2*h + j owns head h (of 4) and N-half j
(4096 of 8192 latent dims). encoder/encoder_v column-parallel, decoder
row-parallel. Per layer: one 2-rank AllReduce (partial yKV within a head
pair, since scores contract over the full head N) and one 8-rank AllReduce
(y = xy @ decoder partial sums into D).

All on-device tensors are fp16 (PE matmuls run fp16 at full rate with fp32
PSUM accumulation; verified ~1.3e-3 rel err vs the fp32 reference).

The RoPE frequency table repeats in pairs (quantize(t,2)), so a host-side
even/odd de-interleave permutation of each core's N slice (baked into
encoder/encoder_v columns and decoder rows) turns rotate_half into two
contiguous halves: qe = xe*c - xo*s, qo = xo*c + xe*s.

The causal mask (strict lower) is applied on the transposed score matrix
P[s,t] = scores[t,s]: Q@Q^T is symmetric, so P comes out of the same
matmuls and the mask becomes strict-upper, which lets the per-s-chunk
matmuls skip the all-zero left region entirely (triangle skip).
"""

import numpy as np

import concourse.bass as bass
import concourse.tile as tile
from concourse import bacc, mybir
from concourse.bass_utils import run_bass_kernel_spmd
from concourse.masks import make_identity

F16 = mybir.dt.float16
BF16 = mybir.dt.bfloat16
F32 = mybir.dt.float32
AF = mybir.ActivationFunctionType
ALU = mybir.AluOpType

B, T, D, NH, VOCAB = 1, 512, 256, 4, 256
N = 8192        # latent dim per head
NL = 4096       # per-core N slice
NPAIR = 2048    # rope pairs per core
NT = NL // 128  # 32 n-tiles per core
N_LAYER = 6
EPS = 1e-5
THETA = 2.0 ** 16
TWO_PI = 2.0 * np.pi
CORES = list(range(8))
PAIR_GROUPS = [[0, 1], [2, 3], [4, 5], [6, 7]]

_STATE = {}


# ---------------------------------------------------------------- host prep

def _ln_np(x):
    m = x.mean(-1, keepdims=True)
    v = ((x - m) ** 2).mean(-1, keepdims=True)
    return (x - m) / np.sqrt(v + EPS)


def _rope_pair_tables():
    """cos/sin at even lanes only (freqs repeat in pairs): [T, N//2] f32."""
    t = np.arange(N, dtype=np.float32)
    q = (np.floor(t / 2.0) * 2.0).astype(np.float32)
    freqs = (1.0 / (THETA ** (q / np.float32(N))) / np.float32(TWO_PI)).astype(
        np.float32
    )
    pos = np.arange(T, dtype=np.float32)
    ang = ((pos[:, None] * freqs[None, :]) % 1.0) * np.float32(TWO_PI)
    cos = np.cos(ang).astype(np.float32)
    sin = np.sin(ang).astype(np.float32)
    return cos[:, ::2], sin[:, ::2]


def _tileize_rows(a, rows_per_tile=128):
    """[n_tiles*128, w] -> [128, n_tiles*w] with free dim = (tile, w)."""
    r, w = a.shape
    nt = r // rows_per_tile
    return np.ascontiguousarray(
        a.reshape(nt, rows_per_tile, w).transpose(1, 0, 2).reshape(rows_per_tile, nt * w)
    )


def _build_in_maps(idx, embed, encoder, encoder_v, decoder, lm_head):
    idx = np.asarray(idx)
    embed = np.asarray(embed, dtype=np.float32)
    encoder = np.asarray(encoder, dtype=np.float32)
    encoder_v = np.asarray(encoder_v, dtype=np.float32)
    decoder = np.asarray(decoder, dtype=np.float32)
    lm_head = np.asarray(lm_head, dtype=np.float32)

    x0 = _ln_np(embed[idx[0]]).astype(np.float16)          # [T, D]
    x_td0 = _tileize_rows(x0)                               # [128, 4*256]
    x_dt0 = _tileize_rows(np.ascontiguousarray(x0.T))       # [128, 2*512]

    cos_p, sin_p = _rope_pair_tables()                      # [T, 4096] f32
    # even lanes first, then odd lanes
    perm = np.concatenate([np.arange(0, NL, 2), np.arange(1, NL, 2)])

    maskd = np.triu(np.ones((128, 128), np.float16), k=1)   # keep s < t
    lmh = _tileize_rows(lm_head.astype(np.float16))         # [128, 2*256]

    in_maps = []
    for c in CORES:
        h, j = c // 2, c % 2
        nsl = slice(j * NL, (j + 1) * NL)
        enc_s = encoder[h][:, nsl][:, perm].astype(np.float16)      # [256, 4096]
        ev_s = encoder_v[h][:, nsl][:, perm].astype(np.float16)
        dec_s = decoder[h * N + j * NL : h * N + (j + 1) * NL][perm].astype(
            np.float16
        )                                                            # [4096, 256]
        kp = slice(j * NPAIR, (j + 1) * NPAIR)
        cos_s = np.ascontiguousarray(cos_p[:, kp].T).astype(np.float16)  # [2048, 512]
        sin_s = np.ascontiguousarray(sin_p[:, kp].T).astype(np.float16)
        in_maps.append(
            {
                "enc0": np.ascontiguousarray(enc_s[:128]),
                "enc1": np.ascontiguousarray(enc_s[128:]),
                "ev0": np.ascontiguousarray(ev_s[:128]),
                "ev1": np.ascontiguousarray(ev_s[128:]),
                "decb": _tileize_rows(dec_s),               # [128, 32*256]
                "cosb": _tileize_rows(cos_s),               # [128, 16*512]
                "sinb": _tileize_rows(sin_s),
                "maskd": maskd,
                "x_td0": x_td0,
                "x_dt0": x_dt0,
                "lmh": lmh,
            }
        )
    return in_maps


# ---------------------------------------------------------------- device code

def _ln_chunk(nc, st, out_f16, in_ap, tc, chunk, epst, eng=None):
    """LN over one free-dim chunk: out = (in - mu) * rstd.

    eng selects the vector-capable engine (default DVE); sqrt is ACT."""
    if eng is None:
        eng = nc.vector
    sl = slice(tc * chunk, (tc + 1) * chunk)
    stats = st.tile([128, 6], F32, tag="st6", name="stats")
    mv = st.tile([128, 2], F32, tag="st2", name="mv")
    eng.bn_stats(out=stats, in_=in_ap[:, sl])
    eng.bn_aggr(out=mv, in_=stats)
    nc.scalar.activation(
        out=mv[:, 1:2], in_=mv[:, 1:2], func=AF.Sqrt, bias=epst, scale=1.0
    )
    eng.reciprocal(out=mv[:, 1:2], in_=mv[:, 1:2])
    eng.tensor_scalar(
        out=out_f16[:, sl],
        in0=in_ap[:, sl],
        scalar1=mv[:, 0:1],
        scalar2=mv[:, 1:2],
        op0=ALU.subtract,
        op1=ALU.mult,
    )


def _transpose_blocks(nc, ps, dst, src, n_tc, n_dc, ident):
    """dst[(dc,t-block)] = src[(tc,d-block)]^T for [128,128] blocks.

    src free = (tc, n_dc*128), dst free = (dc, n_tc*128)."""
    for tc in range(n_tc):
        for dc in range(n_dc):
            tr = ps.tile([128, 128], F16, tag="ps")
            nc.tensor.transpose(
                tr, src[:, tc * (n_dc * 128) + dc * 128 :][:, :128], ident
            )
            nc.scalar.copy(
                out=dst[:, dc * (n_tc * 128) + tc * 128 :][:, :128], in_=tr
            )


def _build_bass():
    nc = bacc.Bacc(None, target_bir_lowering=False, num_devices=len(CORES))

    dp = nc.declare_dram_parameter
    enc0_e = dp("enc0", [128, NL], F16, isOutput=False)
    enc1_e = dp("enc1", [128, NL], F16, isOutput=False)
    ev0_e = dp("ev0", [128, NL], F16, isOutput=False)
    ev1_e = dp("ev1", [128, NL], F16, isOutput=False)
    dec_e = dp("decb", [128, NT * D], F16, isOutput=False)
    cos_e = dp("cosb", [128, 16 * T], F16, isOutput=False)
    sin_e = dp("sinb", [128, 16 * T], F16, isOutput=False)
    mask_e = dp("maskd", [128, 128], F16, isOutput=False)
    xtd_e = dp("x_td0", [128, 4 * D], F16, isOutput=False)
    xdt_e = dp("x_dt0", [128, 2 * T], F16, isOutput=False)
    lmh_e = dp("lmh", [128, 2 * VOCAB], F16, isOutput=False)
    out_e = dp("logits", [T, VOCAB], F32, isOutput=True)

    with tile.TileContext(nc) as tc_:
        pools = [
            tc_.tile_pool(name="wt", bufs=1),
            tc_.tile_pool(name="big", bufs=1),
            tc_.tile_pool(name="xp", bufs=2),
            tc_.tile_pool(name="tmp", bufs=1),
            tc_.tile_pool(name="ys", bufs=3),
            tc_.tile_pool(name="st", bufs=8),
            tc_.tile_pool(name="stg", bufs=1),
            tc_.tile_pool(name="ps", bufs=8, space="PSUM"),
            tc_.tile_pool(name="dram", bufs=2, space="DRAM"),
        ]
        wt, big, xp, tmp, ysp, st, stg, ps, dram = [p.__enter__() for p in pools]
        try:
            _emit(nc, wt, big, xp, tmp, ysp, st, stg, ps, dram,
                  enc0_e, enc1_e, ev0_e, ev1_e, dec_e, cos_e, sin_e, mask_e,
                  xtd_e, xdt_e, lmh_e, out_e)
        finally:
            for p in reversed(pools):
                p.__exit__(None, None, None)
    nc.compile()
    return nc


def _emit(nc, wt, big, xp, tmp, ysp, st, stg, ps, dram,
          enc0_e, enc1_e, ev0_e, ev1_e, dec_e, cos_e, sin_e, mask_e,
          xtd_e, xdt_e, lmh_e, out_e):
    dma = nc.sync.dma_start

    # persistent weights / tables
    enc0 = wt.tile([128, NL], F16, tag="enc0")
    enc1 = wt.tile([128, NL], F16, tag="enc1")
    ev0 = wt.tile([128, NL], F16, tag="ev0")
    ev1 = wt.tile([128, NL], F16, tag="ev1")
    dect = wt.tile([128, NT * D], F16, tag="dect")
    cost = wt.tile([128, 16 * T], F16, tag="cost")
    sint = wt.tile([128, 16 * T], F16, tag="sint")
    maskt = wt.tile([128, 128], F16, tag="maskt")
    lmht = wt.tile([128, 2 * VOCAB], F16, tag="lmht")
    ident = wt.tile([128, 128], F16, tag="ident")
    epst = wt.tile([128, 1], F32, tag="epst")

    xsb = big.tile([128, NT * T], F16, tag="xsb")    # xs then xy, (i, t)
    qrb = big.tile([128, NT * T], F16, tag="qrb")    # roped qs, (i, t)
    Pb = big.tile([128, 4 * T], F16, tag="Pb")       # masked scores^T, (sc, t)

    x_first = xp.tile([128, 4 * D], F16, tag="x_td")
    xd_first = xp.tile([128, 2 * T], F16, tag="x_dt")
    dma(out=x_first, in_=xtd_e[:])
    dma(out=xd_first, in_=xdt_e[:])
    dma(out=enc0, in_=enc0_e[:])
    dma(out=enc1, in_=enc1_e[:])
    dma(out=cost, in_=cos_e[:])
    dma(out=sint, in_=sin_e[:])
    dma(out=maskt, in_=mask_e[:])
    dma(out=ev0, in_=ev0_e[:])
    dma(out=ev1, in_=ev1_e[:])
    dma(out=dect, in_=dec_e[:])
    dma(out=lmht, in_=lmh_e[:])
    nc.vector.memset(epst, EPS)
    make_identity(nc, ident[:])

    # zero regions of P (left of the diagonal block) are written once;
    # every layer only rewrites the diagonal + upper blocks
    for m in range(1, 4):
        nc.vector.memset(Pb[:, m * T : m * T + m * 128], 0.0)

    # warm up the collective path (first-call setup costs ~30us) while the
    # weight DMAs stream in; outputs are never consumed. Buffers match the
    # real per-layer collectives in shape so any size-keyed ring setup warms.
    wup = stg.tile([128, 2 * D], F16, tag="wup")
    nc.vector.memset(wup, 0.0)
    wag_i = dram.tile([128, D], F32, tag="wag_i")
    wag_o = dram.tile([2, 128, D], F32, tag="wag_o")
    war_i = dram.tile([128, 2 * D], F16, tag="war_i")
    war_o = dram.tile([128, 2 * D], F16, tag="war_o")
    dma(out=wag_i[:].bitcast(F16), in_=wup)
    dma(out=war_i, in_=wup)
    nc.gpsimd.collective_compute(
        "AllGather", ALU.bypass, replica_groups=PAIR_GROUPS,
        ins=[wag_i.opt()], outs=[wag_o.opt()],
    )
    nc.gpsimd.collective_compute(
        "AllReduce", ALU.add, replica_groups=[CORES],
        ins=[war_i.opt()], outs=[war_o.opt()],
    )

    # phase 1: xs = relu(x @ enc), out [nl, t] tiles. t-halved (so it can
    # slide into the previous layer's AR2 windows) and (even, odd) lane
    # tiles produced pairwise so rope chunks unblock early. Relu copies
    # alternate ACT/DVE to keep either from becoming the feeder bottleneck.
    order = [k for p in zip(range(16), range(16, NT)) for k in p]

    xsb_r = xsb.rearrange("p (i t) -> p i t", t=T)

    def phase1_half(h, x_dt_ap):
        hsl = slice(h * 256, h * 256 + 256)
        rhs0 = x_dt_ap[:, 0 * T : 1 * T][:, hsl]
        rhs1 = x_dt_ap[:, 1 * T : 2 * T][:, hsl]
        for n in range(16):
            mm = ps.tile([128, 512], F32, tag="ps", name="mm1")
            for k, i in enumerate((n, n + 16)):
                nc.tensor.matmul(
                    out=mm[:, k * 256 : (k + 1) * 256],
                    lhsT=enc0[:, i * 128 : (i + 1) * 128],
                    rhs=rhs0, start=True, stop=False,
                )
                nc.tensor.matmul(
                    out=mm[:, k * 256 : (k + 1) * 256],
                    lhsT=enc1[:, i * 128 : (i + 1) * 128],
                    rhs=rhs1, start=False, stop=True,
                )
            # one batched relu for the (n, n+16) tile pair via a strided AP;
            # every 4th pair goes to DVE to keep ACT from being the feeder
            # bottleneck of rope/P3
            nc.scalar.activation(
                out=xsb_r[:, n : n + 17 : 16, hsl],
                in_=mm.rearrange("p (i t) -> p i t", t=256),
                func=AF.Relu,
            )

    x_td, x_dt = x_first, xd_first
    phase1_half(0, x_dt)
    phase1_half(1, x_dt)
    for _layer in range(N_LAYER):
        # -- phase 2: rope, 4 chunks of 4 tile-pairs, all on DVE (GpSimd
        # sharing the reads poisons DVE SBUF ports — measured 4.4x slowdown).
        # qe = xe*c - xo*s, qo = xo*c + xe*s, all plain tensor_tensor (2x).
        # Full-T chunks on purpose: rope must depend on phase-1 h1 so the
        # scheduler cannot hoist it ahead of the AR-h1 LN chain in the
        # in-order DVE queue (h-split rope measured +10us/layer from that).
        CH = 4 * T  # 2048 columns per chunk
        for c in range(4):
            e_sl = slice(c * CH, (c + 1) * CH)
            o_sl = slice(16 * T + c * CH, 16 * T + (c + 1) * CH)
            tme = tmp.tile([128, CH], F16, tag="tmpe")
            nc.vector.tensor_mul(tme, xsb[:, o_sl], sint[:, e_sl])
            nc.vector.tensor_mul(qrb[:, e_sl], xsb[:, e_sl], cost[:, e_sl])
            nc.vector.tensor_sub(qrb[:, e_sl], qrb[:, e_sl], tme)
            tmo = tmp.tile([128, CH], F16, tag="tmpo")
            nc.vector.tensor_mul(tmo, xsb[:, e_sl], sint[:, e_sl])
            nc.vector.tensor_mul(qrb[:, o_sl], xsb[:, o_sl], cost[:, e_sl])
            nc.vector.tensor_add(qrb[:, o_sl], qrb[:, o_sl], tmo)

        # -- phase 3: P[s,t] = (qr^T qr) masked to s < t (triangle skip).
        # jt streamed in rope-chunk completion order so PE consumption
        # tracks DVE production. Two passes over jt: pass A accumulates
        # t-blocks {0,1}, which is all phase 4 h0 needs (strict lower
        # triangle), so the h0 pair-exchange launches while pass B
        # (t-blocks {2,3}) still runs on the PE.
        P_ps = [ps.tile([128, T], F32, tag="ps", name=f"P_ps{m}") for m in range(4)]
        jt_stream = []
        for c in range(4):
            jt_stream += [4 * c + k for k in range(4)]
            jt_stream += [16 + 4 * c + k for k in range(4)]
        ykv_ps = [ps.tile([128, D], F32, tag="ps", name=f"ykv_ps{m}") for m in range(4)]
        stage1 = stg.tile([128, 4 * D], F16, tag="stg1")
        b1o = []

        def p3_pass(ms):
            for idx_jt, jt in enumerate(jt_stream):
                base = jt * T
                for m in ms:
                    t0 = m * 128
                    nc.tensor.matmul(
                        out=P_ps[m][:, t0:T],
                        lhsT=qrb[:, base + t0 : base + t0 + 128],
                        rhs=qrb[:, base + t0 : base + T],
                        start=(idx_jt == 0), stop=(idx_jt == NT - 1),
                        skip_group_check=True,
                    )
            for m in ms:
                t0 = m * 128
                nc.vector.tensor_mul(
                    Pb[:, m * T + t0 : m * T + t0 + 128],
                    P_ps[m][:, t0 : t0 + 128],
                    maskt,
                )
                if m < 3:
                    nc.scalar.copy(
                        out=Pb[:, m * T + t0 + 128 : (m + 1) * T],
                        in_=P_ps[m][:, t0 + 128 : T],
                    )

        # -- phase 4: yKV partial = P^T-contraction with x (V); the pair
        # reduce is a bypass AllGather (fp16-safe, low floor) + local add.
        # Chunks sc > tcn of P^T are identically zero (strict lower) — skip.
        def p4_half(h):
            for k in range(2):
                tcn = 2 * h + k
                for sc in range(tcn + 1):
                    nc.tensor.matmul(
                        out=ykv_ps[tcn],
                        lhsT=Pb[:, sc * T + tcn * 128 : sc * T + (tcn + 1) * 128],
                        rhs=x_td[:, sc * D : (sc + 1) * D],
                        start=(sc == 0), stop=(sc == tcn),
                        skip_group_check=True,
                    )
                # 1/64 pre-scale keeps the pair-sum inside fp16 range; the
                # LN that follows is scale-invariant so this is exact.
                # On ACT (near-PSUM, lightly loaded) to keep DVE free.
                nc.scalar.activation(
                    out=stage1[:, tcn * D : (tcn + 1) * D],
                    in_=ykv_ps[tcn], func=AF.Copy, scale=0.015625,
                )
            b1i_h = dram.tile([128, D], F32, tag=f"b1i{h}", name=f"b1i{h}")
            b1o_h = dram.tile(
                [2, 128, D], F32, tag=f"b1o{h}", name=f"b1o{h}"
            )
            dma(
                out=b1i_h[:].bitcast(F16),
                in_=stage1[:, h * 2 * D : (h + 1) * 2 * D],
            )
            nc.gpsimd.collective_compute(
                "AllGather", ALU.bypass, replica_groups=PAIR_GROUPS,
                ins=[b1i_h.opt()], outs=[b1o_h.opt()],
            )
            b1o.append(b1o_h)

        p3_pass((0, 1))
        p4_half(0)
        p3_pass((2, 3))
        p4_half(1)

        ykvsum = stg.tile([128, 4 * D], F16, tag="ykvsum")
        agt = stg.tile([128, 2 * D], F16, tag="agt")
        ykv_td = xp.tile([128, 4 * D], F16, tag="ykv_td")
        ykv_dt = xp.tile([128, 2 * T], F16, tag="ykv_dt")
        stage2 = stg.tile([128, 4 * D], F16, tag="stg2")
        b2o = []
        for h in range(2):
            hsl = slice(h * 2 * D, (h + 1) * 2 * D)
            # per-half y-partial accumulators (2 banks held instead of 4 —
            # holding 4 across all of phase 5 capped the mm5 pipeline depth;
            # NOTE: they must be separate tiles — two interleaved
            # accumulation groups inside one PSUM bank corrupt results)
            y_ps_h = [
                ps.tile([128, D], F32, tag="ps", name=f"y_ps{h}{k}")
                for k in range(2)
            ]
            dma(out=ykvsum[:, hsl], in_=b1o[h][0].bitcast(F16))
            dma(out=agt, in_=b1o[h][1].bitcast(F16))
            nc.vector.tensor_add(ykvsum[:, hsl], ykvsum[:, hsl], agt)
            for k in range(2):
                tcn = 2 * h + k
                _ln_chunk(nc, st, ykv_td, ykvsum, tcn, D, epst)
                for dc in range(2):
                    tr = ps.tile([128, 128], F16, tag="ps", name="tr")
                    nc.tensor.transpose(
                        tr, ykv_td[:, tcn * D + dc * 128 :][:, :128], ident
                    )
                    nc.scalar.copy(
                        out=ykv_dt[:, dc * T + tcn * 128 :][:, :128], in_=tr
                    )

            # phase 5 half: ys = relu(yKV @ encv); xy = xs*ys; y += xy @ dec.
            # xy runs once per 4-tile group through a strided 3D AP to
            # amortize DVE op overhead. The dec matmuls trail the ys stream
            # by 2 groups so the relu->xy cross-engine latency (~1.4us of
            # semaphore hops) is hidden behind later ys groups.
            NG = NT // 4

            def ys_group(g):
                # xy = relu(ys_raw) * xs fused into one DVE op straight from
                # PSUM: drops the ACT relu, the ys staging tile, and one
                # cross-engine semaphore hop from the dec feeder chain.
                for h2 in range(2):
                    mm = ps.tile([128, 512], F32, tag="ps", name="mm5")
                    for k2 in range(2):
                        i = 4 * g + 2 * h2 + k2
                        nc.tensor.matmul(
                            out=mm[:, k2 * 256 : (k2 + 1) * 256],
                            lhsT=ev0[:, i * 128 : (i + 1) * 128],
                            rhs=ykv_dt[:, 0 * T + h * 256 :][:, :256],
                            start=True, stop=False,
                        )
                        nc.tensor.matmul(
                            out=mm[:, k2 * 256 : (k2 + 1) * 256],
                            lhsT=ev1[:, i * 128 : (i + 1) * 128],
                            rhs=ykv_dt[:, 1 * T + h * 256 :][:, :256],
                            start=False, stop=True,
                        )
                    i0 = 4 * g + 2 * h2
                    xs_pair = xsb_r[:, i0 : i0 + 2, h * 256 : (h + 1) * 256]
                    nc.vector.scalar_tensor_tensor(
                        out=xs_pair,
                        in0=mm.rearrange("p (i t) -> p i t", t=256),
                        scalar=0.0, in1=xs_pair,
                        op0=ALU.max, op1=ALU.mult,
                    )

            def dec_group(g):
                for k4 in range(4):
                    i = 4 * g + k4
                    for k in range(2):
                        tcn = 2 * h + k
                        nc.tensor.matmul(
                            out=y_ps_h[k],
                            lhsT=xsb[:, i * T + tcn * 128 : i * T + (tcn + 1) * 128],
                            rhs=dect[:, i * D : (i + 1) * D],
                            start=(i == 0), stop=(i == NT - 1),
                            skip_group_check=True,
                        )

            # NOTE: a 2-group dec stagger was tried here and measured
            # +9us/layer — running ys ahead holds more mm5 PSUM banks
            # concurrently, so the ys stream blocks on ACT relu instead.
            for g in range(NG):
                ys_group(g)
                dec_group(g)

            # 8-core AllReduce of this half's y partials (ACT copy: DVE is
            # the busier engine here and ACT sits next to PSUM)
            for k in range(2):
                nc.scalar.copy(
                    out=stage2[:, (2 * h + k) * D : (2 * h + k + 1) * D],
                    in_=y_ps_h[k],
                )
            b2i_h = dram.tile([128, 2 * D], F16, tag=f"b2i{h}", name=f"b2i{h}")
            b2o_h = dram.tile([128, 2 * D], F16, tag=f"b2o{h}", name=f"b2o{h}")
            dma(out=b2i_h, in_=stage2[:, hsl])
            nc.gpsimd.collective_compute(
                "AllReduce", ALU.add, replica_groups=[CORES],
                ins=[b2i_h.opt()], outs=[b2o_h.opt()],
            )
            b2o.append(b2o_h)

        # -- phase 7: y = LN(ysum); x = LN(x + y); refresh x_dt (per half)
        ysum = stg.tile([128, 4 * D], F16, tag="ysum")
        y_ln = xp.tile([128, 4 * D], F16, tag="y_ln")
        z = xp.tile([128, 4 * D], F16, tag="z")
        x_td_new = xp.tile([128, 4 * D], F16, tag="x_td")
        x_dt_new = xp.tile([128, 2 * T], F16, tag="x_dt")
        for h in range(2):
            hsl = slice(h * 2 * D, (h + 1) * 2 * D)
            dma(out=ysum[:, hsl], in_=b2o[h])
            for k in range(2):
                tcn = 2 * h + k
                _ln_chunk(nc, st, y_ln, ysum, tcn, D, epst)
            nc.vector.tensor_add(z[:, hsl], y_ln[:, hsl], x_td[:, hsl])
            for k in range(2):
                tcn = 2 * h + k
                _ln_chunk(nc, st, x_td_new, z, tcn, D, epst)
                for dc in range(2):
                    tr = ps.tile([128, 128], F16, tag="ps", name="trx")
                    nc.tensor.transpose(
                        tr, x_td_new[:, tcn * D + dc * 128 :][:, :128], ident
                    )
                    nc.scalar.copy(
                        out=x_dt_new[:, dc * T + tcn * 128 :][:, :128], in_=tr
                    )
            # next layer's phase-1 half rides in this AR2/tail window
            if _layer < N_LAYER - 1:
                phase1_half(h, x_dt_new)
            else:
                for k in range(2):
                    tcn = 2 * h + k
                    lg = ps.tile([128, VOCAB], F32, tag="ps", name="lg")
                    for dc in range(2):
                        nc.tensor.matmul(
                            out=lg,
                            lhsT=x_dt_new[:, dc * T + tcn * 128 : dc * T + (tcn + 1) * 128],
                            rhs=lmht[:, dc * VOCAB : (dc + 1) * VOCAB],
                            start=(dc == 0), stop=(dc == 1),
                        )
                    lg_sb = ysp.tile([128, VOCAB], F32, tag="lg", name="lg_sb")
                    nc.vector.tensor_copy(out=lg_sb, in_=lg)
                    dma(out=out_e[tcn * 128 : (tcn + 1) * 128, :], in_=lg_sb)
        x_td, x_dt = x_td_new, x_dt_new



# ---------------------------------------------------------------- entry point

def kernel(idx, embed, encoder, encoder_v, decoder, lm_head):
    if "nc" not in _STATE:
        _STATE["nc"] = _build_bass()
    nc = _STATE["nc"]
    in_maps = _build_in_maps(idx, embed, encoder, encoder_v, decoder, lm_head)
    import os

    trace = bool(int(os.environ.get("KERNEL_TRACE", "0")))
    res = run_bass_kernel_spmd(nc, in_maps, core_ids=CORES, trace=trace)
    _STATE["last_results"] = res
    return res.results[0]["logits"].reshape(B, T, VOCAB).astype(np.float32)

